# revision 34
# baseline (speedup 1.0000x reference)
"""Trainium2 Bass kernel for nn_ClassifierGuided (2-modality top-12-of-16 MoE classifier).

Sharding: pure data-parallel over tokens. 2 modalities x 4096 tokens = 8192
tokens; each of the 8 cores owns 1024 tokens of one modality (cores 0-3 ->
modality 0, cores 4-7 -> modality 1) and that modality's full weights.
Dense-eval MoE (all 16 experts computed, sparse gates applied), so no
all-to-all is needed.

v3: expert matmuls run in fp8 (e4m3) with the DoubleRow perf mode (256-row
contraction per instruction at 0.5 cycles/row => 4x the fp32r rate). Experts
are processed in QUADS of 4 (768 h-dims = 3 DoubleRow chunk-pairs). The x
activations / gating / residual / head run in bf16 (exact f32 PSUM
accumulation); gates are quantized to fp8 and broadcast across partitions via
a DRAM roundtrip with partition-step-0 DMAs, laid out per h-chunk so every
gate-multiply is a full [128,512] op. All gb tiles are resident (no reuse
waits). The main loop is a flat software pipeline over (tile, quad): stage1
h-chunks of quad k+1 interleave with stage2 matmul groups of quad k so the PE
never waits on the Act-paced relu chain.

Engine split: PE = matmuls; Act = relu+bias (PSUM->SBUF f32) + gating exp;
Pool = gate-mults (fp8 out, SBUF only); DVE = top-k gating chain + moe drain
(relu+residual, PSUM) + head bias.
"""
import sys

sys.path.insert(0, "/opt/trn_rl_repo")

import numpy as np
import ml_dtypes

import concourse.bass as bass
import concourse.mybir as mybir
import concourse.tile as tile
from concourse import bacc
from concourse.bass_utils import run_bass_kernel_spmd
from concourse.masks import make_identity

# ---- problem sizes (hardcoded per the harness contract) ----
B = 4096           # tokens per modality
D = 768            # model dim
E = 16             # experts
H = 192            # expert hidden
O = 101            # classifier out
KTOP = 12          # top-k experts
NCORES = 8
BC = B // 4        # 1024 tokens per core
DC = D // 128      # 6 d-chunks
NT = 512           # token tile (matmul moving dim)
NTILES = BC // NT  # 2
NQ = 4             # expert quads (4 experts each)
QH = 4 * H         # 768 h-dims per quad = 6 chunks of 128
QCH = QH // 128    # 6 h-chunks per quad
EH128 = E * H // 128  # 24 total h-chunks
F32 = mybir.dt.float32
F32R = mybir.dt.float32r
BF16 = mybir.dt.bfloat16
F8 = mybir.dt.float8e4
DR = mybir.MatmulPerfMode.DoubleRow
NEG_BIG = -1.0e30

_NC_CACHE = {}


def build_nc():
    nc = bacc.Bacc("TRN2", target_bir_lowering=False, debug=False,
                   num_devices=NCORES)

    # ---- DRAM I/O (per-core views; host pre-packs) ----
    # DMA cost in this regime is dominated by a flat per-transfer latency on
    # the issuing queue, so small tensors are packed into single loads.
    PBW = DC * E + DC * O          # bf16 pack: wg | wo  (702 cols)
    PFW = EH128 + 1                # f32 pack: b1 | bo   (25 cols)
    xbf_d = nc.dram_tensor("xbf", [128, DC, BC], BF16, kind="ExternalInput").ap()
    w1_d = nc.dram_tensor("w1", [128, DC, E * H], F8, kind="ExternalInput").ap()
    w2_d = nc.dram_tensor("w2", [128, EH128, D], F8, kind="ExternalInput").ap()
    pbf_d = nc.dram_tensor("pbf", [128, PBW], BF16, kind="ExternalInput").ap()
    pf32_d = nc.dram_tensor("pf32", [128, PFW], F32, kind="ExternalInput").ap()
    b2_d = nc.dram_tensor("b2", [E, D], F8, kind="ExternalInput").ap()
    outT = nc.dram_tensor("outT", [O, BC], F32, kind="ExternalOutput").ap()

    with tile.TileContext(nc) as tc:
        with tc.tile_pool(name="const", bufs=1) as cpool:
            # resident SBUF tensors
            xbf = cpool.tile([128, DC, BC], BF16)      # x (bf16); later holds z
            x8 = cpool.tile([128, DC, BC], F8)         # x (fp8) for W1 matmuls
            w1sb = cpool.tile([128, DC, E * H], F8)
            w2sb = cpool.tile([128, EH128, D], F8)
            pbf = cpool.tile([128, PBW], BF16)         # wg | wo
            pf32 = cpool.tile([128, PFW], F32)         # b1 | bo
            b2sb = cpool.tile([E, D], F8)
            g8 = cpool.tile([E, BC], F8)               # fp8 gates, expert-major
            ident = cpool.tile([128, 128], F32)
            gdram = cpool.tile([E, BC], F8, space="DRAM")

            def wgb(c):        # [128, E] gating weight chunk
                return pbf[:, E * c:E * (c + 1)]

            def wob(c):        # [128, O] head weight chunk
                return pbf[:, DC * E + O * c:DC * E + O * (c + 1)]

            def b1col(m):      # [128, 1] expert-hidden bias column
                return pf32[:, m:m + 1]

            bosb = pf32  # bo lives in column EH128, partitions 0..O-1

            make_identity(nc, ident[:, :])

            # loads ordered by first use / criticality: the gating chain
            # (xbf -> top-k -> g8 flush -> gb6 broadcast) gates the whole t1
            # pipeline, so xbf pieces go first at high priority. SP queue:
            # gating weights + fp8 tensors + gate broadcasts; Act queue:
            # bf16 x + W2 + Wo (all waitless so the Act engine's relu chain
            # is never blocked).
            # SP queue carries the latency-critical chain (xbf -> gate flush
            # -> gblk); Act queue carries only waitless packs + W2 so the
            # exp/relu chain is blocked at most ~7us; Pool/DVE queues carry
            # no DMAs at all.
            # Each queue has ~3 fast DMA slots before per-slot residency
            # (~7us) throttles it, so the 13 loads are spread across queues:
            # SP: xbf t0, w1, xbf t1 | Act: pbf, pf32, w2, b2 | Pool: gate
            # flushes + gblk broadcasts (data-dependent, issue later anyway).
            with tc.high_priority():
                nc.sync.dma_start(out=xbf[:, :, 0:NT], in_=xbf_d[:, :, 0:NT])
                nc.scalar.dma_start(out=pbf[:, :], in_=pbf_d)
                nc.gpsimd.dma_start(out=xbf[:, :, NT:], in_=xbf_d[:, :, NT:])
                nc.sync.dma_start(out=w1sb[:, :, :], in_=w1_d)
                nc.scalar.dma_start(out=pf32[:, :], in_=pf32_d)
                nc.gpsimd.dma_start(out=w2sb[:, :, :], in_=w2_d)
            nc.scalar.dma_start(out=b2sb[:, :], in_=b2_d)
            # x8 is derived on-chip (bf16 -> fp8 copies) instead of a DMA:
            # ready as soon as xbf is, and no DMA queue slot. t0 split
            # DVE/Pool at high priority, t1 later.
            with tc.high_priority():
                for c in range(DC):
                    eng = nc.vector if c % 2 == 0 else nc.gpsimd
                    eng.tensor_copy(x8[:, c, 0:NT], xbf[:, c, 0:NT])
            for c in range(DC):
                eng = nc.vector if c % 2 == 0 else nc.gpsimd
                eng.tensor_copy(x8[:, c, NT:], xbf[:, c, NT:])

            # gblk[p, e, b] = gate of expert e for token b, broadcast across
            # all 128 partitions. Filled by just 3 partition-step-0 DMAs
            # (HWDGE issue slots are ~500ns each, so DMA count matters).
            gblk = cpool.tile([128, E, BC], F8)

            def load_gblk(t):
                fs = slice(NT * t, NT * (t + 1))
                nc.gpsimd.dma_start(
                    out=gblk[:, :, fs],
                    in_=bass.AP(tensor=gdram.tensor, offset=NT * t,
                                ap=[[0, 128], [BC, 16], [1, NT]]))

            # ---------------- gating pass (128-token subtiles) ----------------
            # logits accumulate exactly in f32 PSUM from bf16 inputs; top-12
            # selection + softmax runs in f32 on DVE, gates stored as fp8.
            with tc.tile_pool(name="gps", bufs=4, space="PSUM") as gps, \
                 tc.tile_pool(name="gtp", bufs=2, space="PSUM") as gtp, \
                 tc.tile_pool(name="gsb", bufs=3) as gsb, \
                 tc.high_priority():
                for i in range(BC // 128):
                    ts = slice(128 * i, 128 * (i + 1))
                    lg = gps.tile([128, E], F32, tag="lg")
                    for c in range(DC):
                        nc.tensor.matmul(lg[:, :], xbf[:, c, ts], wgb(c),
                                         start=(c == 0), stop=(c == DC - 1))
                    # top-8 values, then values 9..16 after masking them out
                    # (all reads straight from PSUM)
                    t8a = gsb.tile([128, 8], F32, tag="t8a")
                    nc.vector.max(t8a[:, :], lg[:, :])
                    l2 = gsb.tile([128, E], F32, tag="l2")
                    nc.vector.match_replace(l2[:, :], t8a[:, :], lg[:, :], NEG_BIG)
                    t8b = gsb.tile([128, 8], F32, tag="t8b")
                    nc.vector.max(t8b[:, :], l2[:, :])
                    # softmax over entries >= 12th-largest (t8b[:,3])
                    e16 = gsb.tile([128, E], F32, tag="e16")
                    nc.scalar.activation(e16[:, :], lg[:, :],
                                         mybir.ActivationFunctionType.Exp)
                    em = gsb.tile([128, E], F32, tag="em")
                    ssum = gsb.tile([128, 1], F32, tag="ssum")
                    nc.vector.scalar_tensor_tensor(
                        out=em[:, :], in0=lg[:, :], scalar=t8b[:, 3:4],
                        in1=e16[:, :], op0=mybir.AluOpType.is_ge,
                        op1=mybir.AluOpType.mult, accum_out=ssum[:, :])
                    rinv = gsb.tile([128, 1], F32, tag="rinv")
                    nc.vector.reciprocal(rinv[:, :], ssum[:, :])
                    g = gsb.tile([128, E], F32, tag="g")
                    nc.vector.tensor_scalar_mul(g[:, :], em[:, :], rinv[:, :])
                    # transpose to expert-major, quantize to fp8
                    gt_ps = gtp.tile([E, 128], F32, tag="gt")
                    nc.tensor.transpose(gt_ps[:, :], g[:, :], ident[:, :])
                    nc.vector.tensor_copy(g8[:, ts], gt_ps[:, :])
            # flush each token-half of the gates and start the partition-
            # broadcast reads. Emitted at NORMAL priority: a high-priority
            # DMA that waits on gating data holds an HWDGE slot and
            # head-of-line blocks every other load in the kernel.
            def flush_gates(t):
                fs = slice(NT * t, NT * (t + 1))
                nc.gpsimd.dma_start(out=gdram[:, fs], in_=g8[:, fs])
                load_gblk(t)

            flush_gates(0)

            # ---------------- main loop: flat (tile, quad) pipeline ----------
            with tc.tile_pool(name="moeps", bufs=DC, space="PSUM") as moeps, \
                 tc.tile_pool(name="hps", bufs=2, space="PSUM") as hps, \
                 tc.tile_pool(name="hsb", bufs=4) as hsb, \
                 tc.tile_pool(name="hgpool", bufs=3) as hgpool, \
                 tc.tile_pool(name="opool", bufs=2) as opool:

                moes = {}      # t -> list of 6 moe PSUM tiles
                hgs = {}       # (t, q) -> hg tile
                out_ps = {}    # t -> head PSUM tile

                def s1_chunk(t, q, j):
                    """One h-chunk: 3 DR matmuls + relu(Act) + gate-mult."""
                    ts = slice(NT * t, NT * (t + 1))
                    m = QCH * q + j
                    hcol = 128 * m
                    hps_t = hps.tile([128, NT], F32, tag="h", name="h")
                    for c2 in range(DC // 2):
                        nc.tensor.matmul(
                            hps_t[:, :],
                            w1sb[:, 2 * c2:2 * c2 + 2, hcol:hcol + 128],
                            x8[:, 2 * c2:2 * c2 + 2, ts],
                            start=(c2 == 0), stop=(c2 == DC // 2 - 1),
                            perf_mode=DR)
                    hf = hsb.tile([128, NT], F32, tag="hf", name="hf")
                    nc.scalar.activation(hf[:, :], hps_t[:, :],
                                         mybir.ActivationFunctionType.Relu,
                                         bias=b1col(m))
                    hgq = hgs[(t, q)]
                    if j in (1, 4):
                        # mixed chunk: lower 64 partitions belong to one
                        # expert, upper 64 to the next; split across engines
                        elo = 4 * q + (0 if j == 1 else 2)
                        nc.gpsimd.tensor_tensor(
                            out=hgq[0:64, j, :], in0=hf[0:64, :],
                            in1=gblk[0:64, elo, ts], op=mybir.AluOpType.mult)
                        nc.vector.tensor_tensor(
                            out=hgq[64:128, j, :], in0=hf[64:128, :],
                            in1=gblk[64:128, elo + 1, ts],
                            op=mybir.AluOpType.mult)
                    else:
                        e = 4 * q + {0: 0, 2: 1, 3: 2, 5: 3}[j]
                        eng = nc.gpsimd if j in (0, 3) else nc.vector
                        eng.tensor_tensor(
                            out=hgq[:, j, :], in0=hf[:, :],
                            in1=gblk[:, e, ts], op=mybir.AluOpType.mult)

                def s1_alloc(t, q):
                    hgs[(t, q)] = hgpool.tile([128, QCH, NT], F8, tag="hg",
                                              name="hg")

                def s2_mm(t, q, j2, c):
                    m2 = (QCH // 2) * q + j2
                    nc.tensor.matmul(
                        moes[t][c][:, :],
                        w2sb[:, 2 * m2:2 * m2 + 2, 128 * c:128 * (c + 1)],
                        hgs[(t, q)][:, 2 * j2:2 * j2 + 2, :],
                        start=(q == 0 and j2 == 0), stop=False, perf_mode=DR)

                def finish_chunk(t, c):
                    # z = relu(moe) + x, bf16 in place of x. Alternate the
                    # engine: DVE does it in one scalar_tensor_tensor; for odd
                    # chunks split relu(Act, PSUM->SBUF) + add(Pool, SBUF) so
                    # the close phase isn't paced by a single engine.
                    ts = slice(NT * t, NT * (t + 1))
                    if c % 2 == 0:
                        nc.vector.scalar_tensor_tensor(
                            out=xbf[:, c, ts], in0=moes[t][c][:, :], scalar=0.0,
                            in1=xbf[:, c, ts], op0=mybir.AluOpType.max,
                            op1=mybir.AluOpType.add)
                    else:
                        mt = hsb.tile([128, NT], F32, tag="hf", name="mtmp")
                        nc.scalar.activation(mt[:, :], moes[t][c][:, :],
                                             mybir.ActivationFunctionType.Relu)
                        nc.gpsimd.tensor_tensor(
                            out=xbf[:, c, ts], in0=mt[:, :],
                            in1=xbf[:, c, ts], op=mybir.AluOpType.add)

                def head_chunk(t, c):
                    ts = slice(NT * t, NT * (t + 1))
                    if t not in out_ps:
                        out_ps[t] = hps.tile([O, NT], F32, tag="h",
                                             name="out_ps")
                    nc.tensor.matmul(out_ps[t][:, :], wob(c),
                                     xbf[:, c, ts],
                                     start=(c == 0), stop=(c == DC - 1))

                def s2_groups(t, q, close):
                    """Yield stage2 work as 6 groups (to interleave with the
                    next quad's 6 s1 chunks)."""
                    ts = slice(NT * t, NT * (t + 1))
                    if not close:
                        # j2-major so the last hg chunk-pair is needed late
                        for j2 in range(QCH // 2):
                            for ch in range(2):
                                def grp(j2=j2, ch=ch):
                                    for c in range(3 * ch, 3 * ch + 3):
                                        s2_mm(t, q, j2, c)
                                yield grp
                        return
                    # close: j2=0,1 accumulation first (j2-major), then a
                    # c-major pass with the final pair + b2 + drain + head
                    def grp_a():
                        for c in range(DC):
                            s2_mm(t, q, 0, c)
                    yield grp_a

                    def grp_b():
                        for c in range(DC):
                            s2_mm(t, q, 1, c)
                    yield grp_b

                    for ch in range(4):
                        def grp_c(ch=ch):
                            cs = [(0, 1), (2,), (3, 4), (5,)][ch]
                            for c in cs:
                                s2_mm(t, q, 2, c)
                                nc.tensor.matmul(
                                    moes[t][c][:, :],
                                    b2sb[:, 128 * c:128 * (c + 1)],
                                    g8[:, ts], start=False, stop=True)
                                finish_chunk(t, c)
                                if c >= 2:
                                    head_chunk(t, c - 2)
                            if ch == 3:
                                head_chunk(t, DC - 2)
                                head_chunk(t, DC - 1)
                        yield grp_c

                def emit_out(t):
                    ts = slice(NT * t, NT * (t + 1))
                    osb = opool.tile([O, NT], F32, tag="osb")
                    nc.vector.tensor_scalar_add(osb[:, :], out_ps[t][:, :],
                                                bosb[0:O, EH128:EH128 + 1])
                    nc.sync.dma_start(out=outT[:, ts], in_=osb[:, :])

                seq = [(t, q) for t in range(NTILES) for q in range(NQ)]
                for t in range(NTILES):
                    moes[t] = [moeps.tile([128, NT], F32, tag="moe", name="moe")
                               for _ in range(DC)]
                # prologue: first quad's stage1, un-interleaved
                s1_alloc(0, 0)
                for j in range(QCH):
                    s1_chunk(0, 0, j)
                for idx, (t, q) in enumerate(seq):
                    if idx == 1:
                        flush_gates(1)
                    nxt = seq[idx + 1] if idx + 1 < len(seq) else None
                    groups = list(s2_groups(t, q, close=(q == NQ - 1)))
                    if nxt is not None:
                        s1_alloc(*nxt)
                        for j in range(QCH):
                            s1_chunk(nxt[0], nxt[1], j)
                            groups[j]()
                    else:
                        for grp in groups:
                            grp()
                    if q == NQ - 1:
                        emit_out(t)

    nc.compile()
    return nc


def _pack_core_inputs(x, Wg, W1, b1, W2, b2, Wo, bo, c4):
    """Per-core input dict for one modality's weights + 1024-token slice."""
    f = np.float32
    bf = ml_dtypes.bfloat16
    f8 = ml_dtypes.float8_e4m3
    tok = slice(BC * c4, BC * (c4 + 1))
    xT = np.asarray(x[tok], f).T                       # [D, BC]
    xr = np.ascontiguousarray(
        xT.reshape(DC, 128, BC).transpose(1, 0, 2))    # [128, DC, BC]
    w1f = np.asarray(W1, f).transpose(1, 0, 2).reshape(D, E * H)
    w2f = np.asarray(W2, f).reshape(E * H, D)
    # bf16 pack: [wg (DC*E cols) | wo (DC*O cols)]
    pbf = np.zeros((128, DC * E + DC * O), bf)
    pbf[:, :DC * E] = np.asarray(Wg, f).reshape(DC, 128, E).transpose(
        1, 0, 2).reshape(128, DC * E).astype(bf)
    pbf[:, DC * E:] = np.asarray(Wo, f).reshape(DC, 128, O).transpose(
        1, 0, 2).reshape(128, DC * O).astype(bf)
    # f32 pack: [b1 (EH128 cols) | bo (1 col, partitions 0..O-1)]
    pf32 = np.zeros((128, EH128 + 1), f)
    pf32[:, :EH128] = np.asarray(b1, f).reshape(EH128, 128).T
    pf32[:O, EH128] = np.asarray(bo, f)
    return {
        "xbf": xr.astype(bf),
        "w1": np.ascontiguousarray(
            w1f.reshape(DC, 128, E * H).transpose(1, 0, 2)).astype(f8),
        "w2": np.ascontiguousarray(
            w2f.reshape(EH128, 128, D).transpose(1, 0, 2)).astype(f8),
        "pbf": pbf,
        "pf32": pf32,
        "b2": np.asarray(b2, f).astype(f8),
    }


def run_on_hw(inputs, trace=False, **kw):
    if "nc" not in _NC_CACHE:
        _NC_CACHE["nc"] = build_nc()
    nc = _NC_CACHE["nc"]
    in_maps = []
    for core in range(NCORES):
        i, c4 = divmod(core, 4)
        x = inputs["x0"] if i == 0 else inputs["x1"]
        in_maps.append(_pack_core_inputs(
            x, inputs["Wg"][i], inputs["W1"][i], inputs["b1"][i],
            inputs["W2"][i], inputs["b2"][i], inputs["Wo"][i], inputs["bo"][i], c4))
    res = run_bass_kernel_spmd(nc, in_maps, core_ids=list(range(NCORES)),
                               trace=trace, **kw)
    outs = []
    for i in range(2):
        outs.append(np.concatenate(
            [res.results[4 * i + c]["outT"].T for c in range(4)], axis=0))
    return (outs[0], outs[1]), res


def kernel(**inputs):
    (o0, o1), _ = run_on_hw(inputs)
    return (o0, o1)


# revision 35
# speedup vs baseline: 1.0783x; 1.0783x over previous
"""Trainium2 Bass kernel for nn_ClassifierGuided (2-modality top-12-of-16 MoE classifier).

Sharding: pure data-parallel over tokens. 2 modalities x 4096 tokens = 8192
tokens; each of the 8 cores owns 1024 tokens of one modality (cores 0-3 ->
modality 0, cores 4-7 -> modality 1) and that modality's full weights.
Dense-eval MoE (all 16 experts computed, sparse gates applied), so no
all-to-all is needed.

v3: expert matmuls run in fp8 (e4m3) with the DoubleRow perf mode (256-row
contraction per instruction at 0.5 cycles/row => 4x the fp32r rate). Experts
are processed in QUADS of 4 (768 h-dims = 3 DoubleRow chunk-pairs). The x
activations / gating / residual / head run in bf16 (exact f32 PSUM
accumulation); gates are quantized to fp8 and broadcast across partitions via
a DRAM roundtrip with partition-step-0 DMAs, laid out per h-chunk so every
gate-multiply is a full [128,512] op. All gb tiles are resident (no reuse
waits). The main loop is a flat software pipeline over (tile, quad): stage1
h-chunks of quad k+1 interleave with stage2 matmul groups of quad k so the PE
never waits on the Act-paced relu chain.

Engine split: PE = matmuls; Act = relu+bias (PSUM->SBUF f32) + gating exp;
Pool = gate-mults (fp8 out, SBUF only); DVE = top-k gating chain + moe drain
(relu+residual, PSUM) + head bias.
"""
import sys

sys.path.insert(0, "/opt/trn_rl_repo")

import numpy as np
import ml_dtypes

import concourse.bass as bass
import concourse.mybir as mybir
import concourse.tile as tile
from concourse import bacc
from concourse.bass_utils import run_bass_kernel_spmd
from concourse.masks import make_identity

# ---- problem sizes (hardcoded per the harness contract) ----
B = 4096           # tokens per modality
D = 768            # model dim
E = 16             # experts
H = 192            # expert hidden
O = 101            # classifier out
KTOP = 12          # top-k experts
NCORES = 8
BC = B // 4        # 1024 tokens per core
DC = D // 128      # 6 d-chunks
NT = 512           # token tile (matmul moving dim)
NTILES = BC // NT  # 2
NQ = 4             # expert quads (4 experts each)
QH = 4 * H         # 768 h-dims per quad = 6 chunks of 128
QCH = QH // 128    # 6 h-chunks per quad
EH128 = E * H // 128  # 24 total h-chunks
F32 = mybir.dt.float32
F32R = mybir.dt.float32r
BF16 = mybir.dt.bfloat16
F8 = mybir.dt.float8e4
DR = mybir.MatmulPerfMode.DoubleRow
NEG_BIG = -1.0e30

_NC_CACHE = {}


def build_nc():
    nc = bacc.Bacc("TRN2", target_bir_lowering=False, debug=False,
                   num_devices=NCORES)

    # ---- DRAM I/O (per-core views; host pre-packs) ----
    # DMA cost in this regime is dominated by a flat per-transfer latency on
    # the issuing queue, so small tensors are packed into single loads.
    PBW = DC * E + DC * O          # bf16 pack: wg | wo  (702 cols)
    PFW = EH128 + 1                # f32 pack: b1 | bo   (25 cols)
    xbf_d = nc.dram_tensor("xbf", [128, DC, BC], BF16, kind="ExternalInput").ap()
    w1_d = nc.dram_tensor("w1", [128, DC, E * H], F8, kind="ExternalInput").ap()
    w2_d = nc.dram_tensor("w2", [128, EH128, D], F8, kind="ExternalInput").ap()
    pbf_d = nc.dram_tensor("pbf", [128, PBW], BF16, kind="ExternalInput").ap()
    pf32_d = nc.dram_tensor("pf32", [128, PFW], F32, kind="ExternalInput").ap()
    b2_d = nc.dram_tensor("b2", [E, D], F8, kind="ExternalInput").ap()
    outT = nc.dram_tensor("outT", [O, BC], F32, kind="ExternalOutput").ap()

    with tile.TileContext(nc) as tc:
        with tc.tile_pool(name="const", bufs=1) as cpool:
            # resident SBUF tensors
            xbf = cpool.tile([128, DC, BC], BF16)      # x (bf16); later holds z
            x8 = cpool.tile([128, DC, BC], F8)         # x (fp8) for W1 matmuls
            w1sb = cpool.tile([128, DC, E * H], F8)
            w2sb = cpool.tile([128, EH128, D], F8)
            pbf = cpool.tile([128, PBW], BF16)         # wg | wo
            pf32 = cpool.tile([128, PFW], F32)         # b1 | bo
            b2sb = cpool.tile([E, D], F8)
            g8 = cpool.tile([E, BC], F8)               # fp8 gates, expert-major
            ident = cpool.tile([128, 128], F32)
            gdram = cpool.tile([E, BC], F8, space="DRAM")

            def wgb(c):        # [128, E] gating weight chunk
                return pbf[:, E * c:E * (c + 1)]

            def wob(c):        # [128, O] head weight chunk
                return pbf[:, DC * E + O * c:DC * E + O * (c + 1)]

            def b1col(m):      # [128, 1] expert-hidden bias column
                return pf32[:, m:m + 1]

            bosb = pf32  # bo lives in column EH128, partitions 0..O-1

            make_identity(nc, ident[:, :])

            # loads ordered by first use / criticality: the gating chain
            # (xbf -> top-k -> g8 flush -> gb6 broadcast) gates the whole t1
            # pipeline, so xbf pieces go first at high priority. SP queue:
            # gating weights + fp8 tensors + gate broadcasts; Act queue:
            # bf16 x + W2 + Wo (all waitless so the Act engine's relu chain
            # is never blocked).
            # SP queue carries the latency-critical chain (xbf -> gate flush
            # -> gblk); Act queue carries only waitless packs + W2 so the
            # exp/relu chain is blocked at most ~7us; Pool/DVE queues carry
            # no DMAs at all.
            # Each queue has ~3 fast DMA slots before per-slot residency
            # (~7us) throttles it, so the 13 loads are spread across queues:
            # SP: xbf t0, w1, xbf t1 | Act: pbf, pf32, w2, b2 | Pool: gate
            # flushes + gblk broadcasts (data-dependent, issue later anyway).
            with tc.high_priority():
                nc.sync.dma_start(out=xbf[:, :, 0:NT], in_=xbf_d[:, :, 0:NT])
                nc.scalar.dma_start(out=pbf[:, :], in_=pbf_d)
                nc.gpsimd.dma_start(out=xbf[:, :, NT:], in_=xbf_d[:, :, NT:])
                nc.sync.dma_start(out=w1sb[:, :, :], in_=w1_d)
                nc.scalar.dma_start(out=pf32[:, :], in_=pf32_d)
                nc.gpsimd.dma_start(out=w2sb[:, :, :], in_=w2_d)
            nc.scalar.dma_start(out=b2sb[:, :], in_=b2_d)
            # x8 is derived on-chip (bf16 -> fp8 copies) instead of a DMA:
            # ready as soon as xbf is, and no DMA queue slot. t0 split
            # DVE/Pool at high priority, t1 later.
            with tc.high_priority():
                for c in range(DC):
                    eng = nc.vector if c % 2 == 0 else nc.gpsimd
                    eng.tensor_copy(x8[:, c, 0:NT], xbf[:, c, 0:NT])
            for c in range(DC):
                nc.vector.tensor_copy(x8[:, c, NT:], xbf[:, c, NT:])

            # gblk[p, e, b] = gate of expert e for token b, broadcast across
            # all 128 partitions. Filled by just 3 partition-step-0 DMAs
            # (HWDGE issue slots are ~500ns each, so DMA count matters).
            gblk = cpool.tile([128, E, BC], F8)

            def load_gblk(t):
                fs = slice(NT * t, NT * (t + 1))
                nc.gpsimd.dma_start(
                    out=gblk[:, :, fs],
                    in_=bass.AP(tensor=gdram.tensor, offset=NT * t,
                                ap=[[0, 128], [BC, 16], [1, NT]]))

            # ---------------- gating pass (128-token subtiles) ----------------
            # logits accumulate exactly in f32 PSUM from bf16 inputs; top-12
            # selection + softmax runs in f32 on DVE, gates stored as fp8.
            with tc.tile_pool(name="gps", bufs=4, space="PSUM") as gps, \
                 tc.tile_pool(name="gtp", bufs=2, space="PSUM") as gtp, \
                 tc.tile_pool(name="gsb", bufs=3) as gsb, \
                 tc.high_priority():
                for i in range(BC // 128):
                    ts = slice(128 * i, 128 * (i + 1))
                    lg = gps.tile([128, E], F32, tag="lg")
                    for c in range(DC):
                        nc.tensor.matmul(lg[:, :], xbf[:, c, ts], wgb(c),
                                         start=(c == 0), stop=(c == DC - 1))
                    # top-8 values, then values 9..16 after masking them out
                    # (all reads straight from PSUM)
                    t8a = gsb.tile([128, 8], F32, tag="t8a")
                    nc.vector.max(t8a[:, :], lg[:, :])
                    l2 = gsb.tile([128, E], F32, tag="l2")
                    nc.vector.match_replace(l2[:, :], t8a[:, :], lg[:, :], NEG_BIG)
                    t8b = gsb.tile([128, 8], F32, tag="t8b")
                    nc.vector.max(t8b[:, :], l2[:, :])
                    # softmax over entries >= 12th-largest (t8b[:,3])
                    e16 = gsb.tile([128, E], F32, tag="e16")
                    nc.scalar.activation(e16[:, :], lg[:, :],
                                         mybir.ActivationFunctionType.Exp)
                    em = gsb.tile([128, E], F32, tag="em")
                    ssum = gsb.tile([128, 1], F32, tag="ssum")
                    nc.vector.scalar_tensor_tensor(
                        out=em[:, :], in0=lg[:, :], scalar=t8b[:, 3:4],
                        in1=e16[:, :], op0=mybir.AluOpType.is_ge,
                        op1=mybir.AluOpType.mult, accum_out=ssum[:, :])
                    rinv = gsb.tile([128, 1], F32, tag="rinv")
                    nc.vector.reciprocal(rinv[:, :], ssum[:, :])
                    g = gsb.tile([128, E], F32, tag="g")
                    nc.vector.tensor_scalar_mul(g[:, :], em[:, :], rinv[:, :])
                    # transpose to expert-major, quantize to fp8
                    gt_ps = gtp.tile([E, 128], F32, tag="gt")
                    nc.tensor.transpose(gt_ps[:, :], g[:, :], ident[:, :])
                    nc.vector.tensor_copy(g8[:, ts], gt_ps[:, :])
            # flush each token-half of the gates and start the partition-
            # broadcast reads. Emitted at NORMAL priority: a high-priority
            # DMA that waits on gating data holds an HWDGE slot and
            # head-of-line blocks every other load in the kernel.
            def flush_gates(t):
                fs = slice(NT * t, NT * (t + 1))
                nc.gpsimd.dma_start(out=gdram[:, fs], in_=g8[:, fs])
                load_gblk(t)

            flush_gates(0)

            # ---------------- main loop: flat (tile, quad) pipeline ----------
            with tc.tile_pool(name="moeps", bufs=DC, space="PSUM") as moeps, \
                 tc.tile_pool(name="hps", bufs=2, space="PSUM") as hps, \
                 tc.tile_pool(name="hsb", bufs=4) as hsb, \
                 tc.tile_pool(name="hgpool", bufs=3) as hgpool, \
                 tc.tile_pool(name="opool", bufs=2) as opool:

                moes = {}      # t -> list of 6 moe PSUM tiles
                hgs = {}       # (t, q) -> hg tile
                out_ps = {}    # t -> head PSUM tile

                def s1_chunk(t, q, j):
                    """One h-chunk: 3 DR matmuls + relu(Act) + gate-mult."""
                    ts = slice(NT * t, NT * (t + 1))
                    m = QCH * q + j
                    hcol = 128 * m
                    hps_t = hps.tile([128, NT], F32, tag="h", name="h")
                    for c2 in range(DC // 2):
                        nc.tensor.matmul(
                            hps_t[:, :],
                            w1sb[:, 2 * c2:2 * c2 + 2, hcol:hcol + 128],
                            x8[:, 2 * c2:2 * c2 + 2, ts],
                            start=(c2 == 0), stop=(c2 == DC // 2 - 1),
                            perf_mode=DR)
                    hf = hsb.tile([128, NT], F32, tag="hf", name="hf")
                    nc.scalar.activation(hf[:, :], hps_t[:, :],
                                         mybir.ActivationFunctionType.Relu,
                                         bias=b1col(m))
                    hgq = hgs[(t, q)]
                    if j in (1, 4):
                        # mixed chunk: lower 64 partitions belong to one
                        # expert, upper 64 to the next; split across engines
                        elo = 4 * q + (0 if j == 1 else 2)
                        nc.gpsimd.tensor_tensor(
                            out=hgq[0:64, j, :], in0=hf[0:64, :],
                            in1=gblk[0:64, elo, ts], op=mybir.AluOpType.mult)
                        nc.vector.tensor_tensor(
                            out=hgq[64:128, j, :], in0=hf[64:128, :],
                            in1=gblk[64:128, elo + 1, ts],
                            op=mybir.AluOpType.mult)
                    else:
                        e = 4 * q + {0: 0, 2: 1, 3: 2, 5: 3}[j]
                        eng = nc.gpsimd if j in (0, 3) else nc.vector
                        eng.tensor_tensor(
                            out=hgq[:, j, :], in0=hf[:, :],
                            in1=gblk[:, e, ts], op=mybir.AluOpType.mult)

                def s1_alloc(t, q):
                    hgs[(t, q)] = hgpool.tile([128, QCH, NT], F8, tag="hg",
                                              name="hg")

                def s2_mm(t, q, j2, c):
                    m2 = (QCH // 2) * q + j2
                    nc.tensor.matmul(
                        moes[t][c][:, :],
                        w2sb[:, 2 * m2:2 * m2 + 2, 128 * c:128 * (c + 1)],
                        hgs[(t, q)][:, 2 * j2:2 * j2 + 2, :],
                        start=(q == 0 and j2 == 0), stop=False, perf_mode=DR)

                def finish_chunk(t, c):
                    # z = relu(moe) + x, bf16 in place of x. Alternate the
                    # engine: DVE does it in one scalar_tensor_tensor; for odd
                    # chunks split relu(Act, PSUM->SBUF) + add(Pool, SBUF) so
                    # the close phase isn't paced by a single engine.
                    ts = slice(NT * t, NT * (t + 1))
                    if c % 2 == 0:
                        nc.vector.scalar_tensor_tensor(
                            out=xbf[:, c, ts], in0=moes[t][c][:, :], scalar=0.0,
                            in1=xbf[:, c, ts], op0=mybir.AluOpType.max,
                            op1=mybir.AluOpType.add)
                    else:
                        mt = hsb.tile([128, NT], F32, tag="hf", name="mtmp")
                        nc.scalar.activation(mt[:, :], moes[t][c][:, :],
                                             mybir.ActivationFunctionType.Relu)
                        nc.gpsimd.tensor_tensor(
                            out=xbf[:, c, ts], in0=mt[:, :],
                            in1=xbf[:, c, ts], op=mybir.AluOpType.add)

                def head_chunk(t, c):
                    ts = slice(NT * t, NT * (t + 1))
                    if t not in out_ps:
                        out_ps[t] = hps.tile([O, NT], F32, tag="h",
                                             name="out_ps")
                    nc.tensor.matmul(out_ps[t][:, :], wob(c),
                                     xbf[:, c, ts],
                                     start=(c == 0), stop=(c == DC - 1))

                def s2_groups(t, q, close):
                    """Yield stage2 work as 6 groups (to interleave with the
                    next quad's 6 s1 chunks)."""
                    ts = slice(NT * t, NT * (t + 1))
                    if not close:
                        # j2-major so the last hg chunk-pair is needed late
                        for j2 in range(QCH // 2):
                            for ch in range(2):
                                def grp(j2=j2, ch=ch):
                                    for c in range(3 * ch, 3 * ch + 3):
                                        s2_mm(t, q, j2, c)
                                yield grp
                        return
                    # close: j2=0,1 accumulation first (j2-major), then a
                    # c-major pass with the final pair + b2 + drain + head
                    def grp_a():
                        for c in range(DC):
                            s2_mm(t, q, 0, c)
                    yield grp_a

                    def grp_b():
                        for c in range(DC):
                            s2_mm(t, q, 1, c)
                    yield grp_b

                    for ch in range(4):
                        def grp_c(ch=ch):
                            cs = [(0, 1), (2,), (3, 4), (5,)][ch]
                            for c in cs:
                                s2_mm(t, q, 2, c)
                                nc.tensor.matmul(
                                    moes[t][c][:, :],
                                    b2sb[:, 128 * c:128 * (c + 1)],
                                    g8[:, ts], start=False, stop=True)
                                finish_chunk(t, c)
                                if c >= 2:
                                    head_chunk(t, c - 2)
                            if ch == 3:
                                head_chunk(t, DC - 2)
                                head_chunk(t, DC - 1)
                        yield grp_c

                def emit_out(t):
                    ts = slice(NT * t, NT * (t + 1))
                    osb = opool.tile([O, NT], F32, tag="osb")
                    nc.vector.tensor_scalar_add(osb[:, :], out_ps[t][:, :],
                                                bosb[0:O, EH128:EH128 + 1])
                    nc.sync.dma_start(out=outT[:, ts], in_=osb[:, :])

                seq = [(t, q) for t in range(NTILES) for q in range(NQ)]
                for t in range(NTILES):
                    moes[t] = [moeps.tile([128, NT], F32, tag="moe", name="moe")
                               for _ in range(DC)]
                # prologue: first quad's stage1, un-interleaved
                s1_alloc(0, 0)
                for j in range(QCH):
                    s1_chunk(0, 0, j)
                for idx, (t, q) in enumerate(seq):
                    if idx == 1:
                        flush_gates(1)
                    nxt = seq[idx + 1] if idx + 1 < len(seq) else None
                    groups = list(s2_groups(t, q, close=(q == NQ - 1)))
                    if nxt is not None:
                        s1_alloc(*nxt)
                        for j in range(QCH):
                            s1_chunk(nxt[0], nxt[1], j)
                            groups[j]()
                    else:
                        for grp in groups:
                            grp()
                    if q == NQ - 1:
                        emit_out(t)

    nc.compile()
    return nc


def _pack_core_inputs(x, Wg, W1, b1, W2, b2, Wo, bo, c4):
    """Per-core input dict for one modality's weights + 1024-token slice."""
    f = np.float32
    bf = ml_dtypes.bfloat16
    f8 = ml_dtypes.float8_e4m3
    tok = slice(BC * c4, BC * (c4 + 1))
    xT = np.asarray(x[tok], f).T                       # [D, BC]
    xr = np.ascontiguousarray(
        xT.reshape(DC, 128, BC).transpose(1, 0, 2))    # [128, DC, BC]
    w1f = np.asarray(W1, f).transpose(1, 0, 2).reshape(D, E * H)
    w2f = np.asarray(W2, f).reshape(E * H, D)
    # bf16 pack: [wg (DC*E cols) | wo (DC*O cols)]
    pbf = np.zeros((128, DC * E + DC * O), bf)
    pbf[:, :DC * E] = np.asarray(Wg, f).reshape(DC, 128, E).transpose(
        1, 0, 2).reshape(128, DC * E).astype(bf)
    pbf[:, DC * E:] = np.asarray(Wo, f).reshape(DC, 128, O).transpose(
        1, 0, 2).reshape(128, DC * O).astype(bf)
    # f32 pack: [b1 (EH128 cols) | bo (1 col, partitions 0..O-1)]
    pf32 = np.zeros((128, EH128 + 1), f)
    pf32[:, :EH128] = np.asarray(b1, f).reshape(EH128, 128).T
    pf32[:O, EH128] = np.asarray(bo, f)
    return {
        "xbf": xr.astype(bf),
        "w1": np.ascontiguousarray(
            w1f.reshape(DC, 128, E * H).transpose(1, 0, 2)).astype(f8),
        "w2": np.ascontiguousarray(
            w2f.reshape(EH128, 128, D).transpose(1, 0, 2)).astype(f8),
        "pbf": pbf,
        "pf32": pf32,
        "b2": np.asarray(b2, f).astype(f8),
    }


def run_on_hw(inputs, trace=False, **kw):
    if "nc" not in _NC_CACHE:
        _NC_CACHE["nc"] = build_nc()
    nc = _NC_CACHE["nc"]
    in_maps = []
    for core in range(NCORES):
        i, c4 = divmod(core, 4)
        x = inputs["x0"] if i == 0 else inputs["x1"]
        in_maps.append(_pack_core_inputs(
            x, inputs["Wg"][i], inputs["W1"][i], inputs["b1"][i],
            inputs["W2"][i], inputs["b2"][i], inputs["Wo"][i], inputs["bo"][i], c4))
    res = run_bass_kernel_spmd(nc, in_maps, core_ids=list(range(NCORES)),
                               trace=trace, **kw)
    outs = []
    for i in range(2):
        outs.append(np.concatenate(
            [res.results[4 * i + c]["outT"].T for c in range(4)], axis=0))
    return (outs[0], outs[1]), res


def kernel(**inputs):
    (o0, o1), _ = run_on_hw(inputs)
    return (o0, o1)
